# revision 1
# baseline (speedup 1.0000x reference)
"""EquivariantTransformerBlock on 8 TRN2 NeuronCores.

Strategy (edge-partitioned, per sharding hint):
  - Host: gather src/dst node features per edge, fold the constant MLP
    gates (edge_attr_s == const) + all normalizations into the logit
    weights, shard 320k edges across 8 cores.
  - Device (per core, Bass/Tile): per-edge dense compute -> logit[E,4],
    val_s[E,48], val_v[E,144].  4 PE matmuls + DVE tensor-product math
    per 128-edge tile.
  - Host: segment softmax over edge_dst (max seeded at 0), value
    weighting, segment sums, small output linears.
"""

import math
import numpy as np

N, E = 10000, 320000
F0, F1 = 32, 16
K = F0 + F1          # 48
H = 4
HID = 64
SQRT3 = math.sqrt(3.0)
FAN_SQRT = 48.0      # sqrt(F0*K + F1*K) = sqrt(2304)
NCORES = 8
ES = E // NCORES     # 40000
ES_PAD = ((ES + 127) // 128) * 128   # 40064
NTILES = ES_PAD // 128

LAST_EXEC_NS = None  # stashed for test harness


def _gelu(x):
    # jax.nn.gelu default: tanh approximation
    return 0.5 * x * (1.0 + np.tanh(np.sqrt(2.0 / np.pi) * (x + 0.044715 * x ** 3)))


def _mlp_np(y0, W1, W2, W3):
    h = _gelu(y0 @ W1)
    h = _gelu(h @ W2 / np.sqrt(float(HID)))
    return h @ W3 / np.sqrt(float(HID))


def _build_nc():
    import concourse.bass as bass
    import concourse.bacc as bacc
    import concourse.mybir as mybir
    import concourse.tile as tile

    dt = mybir.dt.float32
    nc = bacc.Bacc(None)

    src_s_d = nc.declare_dram_parameter("src_s", [ES_PAD, 32], dt, isOutput=False)
    src_v_d = nc.declare_dram_parameter("src_v", [ES_PAD, 48], dt, isOutput=False)
    r3_d = nc.declare_dram_parameter("r3", [ES_PAD, 3], dt, isOutput=False)
    qsT_d = nc.declare_dram_parameter("qsT", [32, ES_PAD], dt, isOutput=False)
    qvT_d = nc.declare_dram_parameter("qvT", [48, ES_PAD], dt, isOutput=False)
    W0f_d = nc.declare_dram_parameter("W0f", [32, 192], dt, isOutput=False)
    W1f_d = nc.declare_dram_parameter("W1f", [48, 192], dt, isOutput=False)
    gvs_d = nc.declare_dram_parameter("gvs", [128, 48], dt, isOutput=False)
    gvv_d = nc.declare_dram_parameter("gvv", [128, 144], dt, isOutput=False)
    logit_d = nc.declare_dram_parameter("logit", [ES_PAD, 4], dt, isOutput=True)
    vals_d = nc.declare_dram_parameter("val_s", [ES_PAD, 48], dt, isOutput=True)
    valv_d = nc.declare_dram_parameter("val_v", [ES_PAD, 144], dt, isOutput=True)

    X = mybir.AxisListType.X

    with tile.TileContext(nc) as tc:
        with (
            tc.tile_pool(name="const", bufs=1) as cp,
            tc.tile_pool(name="io", bufs=3) as iop,
            tc.tile_pool(name="work", bufs=3) as wp,
            tc.tile_pool(name="psum", bufs=2, space=bass.MemorySpace.PSUM) as pp,
        ):
            W0f_t = cp.tile([32, 192], dt, tag="w0")
            nc.sync.dma_start(W0f_t[:], W0f_d[:])
            W1f_t = []
            for c in range(3):
                w1c = cp.tile([16, 192], dt, tag=f"w1_{c}")
                nc.sync.dma_start(w1c[:], W1f_d[16 * c:16 * c + 16, :])
                W1f_t.append(w1c)
            gvs_t = cp.tile([128, 48], dt, tag="gvs")
            nc.sync.dma_start(gvs_t[:], gvs_d[:])
            gvv_t = cp.tile([128, 144], dt, tag="gvv")
            nc.sync.dma_start(gvv_t[:], gvv_d[:])

            for t in range(NTILES):
                sl = slice(t * 128, (t + 1) * 128)
                ss = iop.tile([128, 32], dt, tag="ss")
                nc.sync.dma_start(ss[:], src_s_d[sl, :])
                sv = iop.tile([128, 48], dt, tag="sv")
                nc.sync.dma_start(sv[:], src_v_d[sl, :])
                rr = iop.tile([128, 3], dt, tag="rr")
                nc.sync.dma_start(rr[:], r3_d[sl, :])
                qs = iop.tile([32, 128], dt, tag="qs")
                nc.sync.dma_start(qs[:], qsT_d[:, sl])
                qv = []
                for c in range(3):
                    qvc = iop.tile([16, 128], dt, tag=f"qv{c}")
                    nc.sync.dma_start(qvc[:], qvT_d[16 * c:16 * c + 16, sl])
                    qv.append(qvc)

                # B0[e,(h,j)] = sum_i q_s[e,i] * W0f[i,(h,j)]
                B0 = pp.tile([128, 192], dt, tag="B0")
                nc.tensor.matmul(B0[:], qs[:], W0f_t[:])
                # D1_c[e,(h,j)] = sum_i q_v[e,i,c] * W1f[i,(h,j)]
                D = []
                for c in range(3):
                    Dc = pp.tile([128, 192], dt, tag=f"D{c}")
                    nc.tensor.matmul(Dc[:], qv[c][:], W1f_t[c][:])
                    D.append(Dc)

                # o1s = [src_s | dot(src_v, r)]   (raw dot; 1/sqrt3 folded in W/gates)
                o1s = wp.tile([128, 48], dt, tag="o1s")
                nc.vector.tensor_copy(o1s[:, 0:32], ss[:])
                dotv = wp.tile([128, 48], dt, tag="dotv")
                sv3 = sv[:].rearrange("p (f c) -> p f c", c=3)
                rr_b16 = rr[:].rearrange("p c -> p () c").to_broadcast((128, 16, 3))
                nc.vector.tensor_mul(dotv[:].rearrange("p (f c) -> p f c", c=3), sv3, rr_b16)
                nc.vector.reduce_sum(
                    o1s[:, 32:48], dotv[:].rearrange("p (f c) -> p f c", c=3), axis=X
                )

                # o1v = [src_v | src_s x r]   layout (j, c), j-major
                o1v = wp.tile([128, 144], dt, tag="o1v")
                nc.vector.tensor_copy(o1v[:, 0:48], sv[:])
                ss_b = ss[:].rearrange("p f -> p f ()").to_broadcast((128, 32, 3))
                rr_b32 = rr[:].rearrange("p c -> p () c").to_broadcast((128, 32, 3))
                nc.vector.tensor_mul(
                    o1v[:, 48:144].rearrange("p (f c) -> p f c", c=3), ss_b, rr_b32
                )

                # T[e,h,:] = [B0_h * o1s | D1_c_h * o1v_c ...] ; logit = sum over last
                Tt = wp.tile([128, 768], dt, tag="Tt")
                Tt3 = Tt[:].rearrange("p (h j) -> p h j", h=4)
                B03 = B0[:].rearrange("p (h j) -> p h j", h=4)
                o1s_b = o1s[:].rearrange("p j -> p () j").to_broadcast((128, 4, 48))
                nc.vector.tensor_mul(Tt3[:, :, 0:48], B03, o1s_b)
                o1v3 = o1v[:].rearrange("p (j c) -> p j c", c=3)
                for c in range(3):
                    Dc3 = D[c][:].rearrange("p (h j) -> p h j", h=4)
                    o1vc_b = (
                        o1v3[:, :, c].rearrange("p j -> p () j").to_broadcast((128, 4, 48))
                    )
                    nc.vector.tensor_mul(
                        Tt3[:, :, 48 + 48 * c:96 + 48 * c], Dc3, o1vc_b
                    )
                lg = wp.tile([128, 4], dt, tag="lg")
                nc.vector.reduce_sum(
                    lg[:], Tt[:].rearrange("p (h j) -> p h j", h=4), axis=X
                )

                # gated values (gpsimd to offload DVE)
                vst = wp.tile([128, 48], dt, tag="vst")
                nc.vector.tensor_mul(vst[:], o1s[:], gvs_t[:])
                vvt = wp.tile([128, 144], dt, tag="vvt")
                nc.vector.tensor_mul(vvt[:], o1v[:], gvv_t[:])

                nc.sync.dma_start(logit_d[sl, :], lg[:])
                nc.sync.dma_start(vals_d[sl, :], vst[:])
                nc.sync.dma_start(valv_d[sl, :], vvt[:])
    nc.compile()
    return nc


_NC_CACHE = None


def kernel(edge_src, edge_dst, edge_weight_cutoff, edge_attr_s, edge_attr_v,
           node_s, node_v, Wk1, Wk2, Wk3, Wv1, Wv2, Wv3, Wlog0, Wlog1,
           Wout0, Wout1):
    global LAST_EXEC_NS, _NC_CACHE
    from concourse.bass_utils import run_bass_kernel_spmd

    f32 = np.float32
    edge_src = np.asarray(edge_src).astype(np.int64)
    edge_dst = np.asarray(edge_dst).astype(np.int64)
    cutoff = np.asarray(edge_weight_cutoff, dtype=f32)
    ea_s = np.asarray(edge_attr_s, dtype=f32)
    r = np.asarray(edge_attr_v, dtype=f32)
    node_s = np.asarray(node_s, dtype=f32)
    node_v = np.asarray(node_v, dtype=f32)
    Wlog0 = np.asarray(Wlog0, dtype=f32)
    Wlog1 = np.asarray(Wlog1, dtype=f32)
    Wout0 = np.asarray(Wout0, dtype=f32)
    Wout1 = np.asarray(Wout1, dtype=f32)

    # MLP gates: edge_attr_s is constant (ones) -> one gate vector
    u = np.unique(ea_s[:, 0])
    assert u.size == 1, "non-constant edge_attr_s unsupported by device path"
    y0 = u.reshape(1, 1).astype(np.float64)
    gk = _mlp_np(y0, np.asarray(Wk1, np.float64), np.asarray(Wk2, np.float64),
                 np.asarray(Wk3, np.float64))[0]
    gv = _mlp_np(y0, np.asarray(Wv1, np.float64), np.asarray(Wv2, np.float64),
                 np.asarray(Wv3, np.float64))[0]

    # fold gates + norms into logit weights: W0f[i, h*48+j], W1f[16c+i, h*48+j]
    scale = 1.0 / FAN_SQRT
    jfac = np.where(np.arange(K) >= F0, 1.0 / SQRT3, 1.0)  # dot part of o1s
    W0f = (Wlog0.astype(np.float64).transpose(0, 2, 1)
           * (gk[:K] * jfac * scale)[None, None, :]).reshape(32, 192)
    W1f_1 = (Wlog1.astype(np.float64).transpose(0, 2, 1)
             * (gk[K:] * scale / SQRT3)[None, None, :]).reshape(16, 192)
    W1f = np.vstack([W1f_1, W1f_1, W1f_1])
    gvs = np.broadcast_to((gv[:K] * jfac)[None, :], (128, K)).copy()
    gvv = np.broadcast_to(np.repeat(gv[K:], 3)[None, :], (128, 144)).copy()

    W0f = np.ascontiguousarray(W0f, dtype=f32)
    W1f = np.ascontiguousarray(W1f, dtype=f32)
    gvs = np.ascontiguousarray(gvs, dtype=f32)
    gvv = np.ascontiguousarray(gvv, dtype=f32)

    # shard + gather
    in_maps = []
    for s in range(NCORES):
        sl = slice(s * ES, (s + 1) * ES)
        esrc, edst = edge_src[sl], edge_dst[sl]
        pad = ES_PAD - ES

        def p(a):  # pad rows with zeros
            return np.pad(a, ((0, pad),) + ((0, 0),) * (a.ndim - 1)) if pad else a

        src_s = p(node_s[esrc])                      # [ES_PAD,32]
        src_v = p(node_v[esrc].reshape(ES, 48))      # [ES_PAD,48]
        r3 = p(r[sl])                                # [ES_PAD,3]
        q_s = p(node_s[edst])
        q_v = p(node_v[edst])                        # [ES_PAD,16,3]
        qsT = np.ascontiguousarray(q_s.T)            # [32,ES_PAD]
        qvT = np.ascontiguousarray(
            q_v.transpose(2, 1, 0).reshape(48, ES_PAD))  # rows 16c+i
        in_maps.append(dict(src_s=np.ascontiguousarray(src_s),
                            src_v=np.ascontiguousarray(src_v),
                            r3=np.ascontiguousarray(r3),
                            qsT=qsT, qvT=qvT,
                            W0f=W0f, W1f=W1f, gvs=gvs, gvv=gvv))

    if _NC_CACHE is None:
        _NC_CACHE = _build_nc()
    import time as _time
    _t0 = _time.time()
    res = run_bass_kernel_spmd(_NC_CACHE, in_maps, core_ids=list(range(NCORES)))
    LAST_EXEC_NS = res.exec_time_ns
    if LAST_EXEC_NS is None:  # no NTFF hook in this container: wall-clock proxy
        LAST_EXEC_NS = int((_time.time() - _t0) * 1e9)

    logit = np.concatenate([res.results[s]["logit"][:ES] for s in range(NCORES)], 0)
    val_s = np.concatenate([res.results[s]["val_s"][:ES] for s in range(NCORES)], 0)
    val_v = np.concatenate([res.results[s]["val_v"][:ES] for s in range(NCORES)], 0)

    # segment softmax over edge_dst, max seeded at 0
    m = np.zeros((N, H), f32)
    np.maximum.at(m, edge_dst, logit)
    ew = cutoff[:, None] * np.exp(logit - m[edge_dst])
    z = np.zeros((N, H), f32)
    for h in range(H):
        z[:, h] = np.bincount(edge_dst, weights=ew[:, h], minlength=N)
    z[z == 0.0] = 1.0
    alpha = ew / z[edge_dst]
    w = np.sqrt(np.maximum(alpha, 0.0)).astype(f32)          # [E,4]

    KH = K // H  # 12
    vs = (val_s.reshape(E, H, KH) * w[:, :, None]).reshape(E, K)
    vv = (val_v.reshape(E, H, KH, 3) * w[:, :, None, None]).reshape(E, 144)

    ns = np.empty((N, K), f32)
    for j in range(K):
        ns[:, j] = np.bincount(edge_dst, weights=vs[:, j], minlength=N)
    nv = np.empty((N, 144), f32)
    for j in range(144):
        nv[:, j] = np.bincount(edge_dst, weights=vv[:, j], minlength=N)

    out_s = ns @ Wout0 / np.sqrt(float(K))                    # [N,32]
    out_v = np.einsum("nfc,fg->ngc", nv.reshape(N, K, 3), Wout1) / np.sqrt(float(K))
    return np.concatenate([out_s, out_v.reshape(N, 48)], axis=1).astype(f32)



# revision 12
# speedup vs baseline: 3.3032x; 3.3032x over previous
"""EquivariantTransformerBlock on 8 TRN2 NeuronCores.

Strategy (v2, transfer-minimizing):
  - Host: sort edges by dst, split dsts into 8 contiguous ranges (~E/8 edges
    each) -> one range per core, so each dst's segment lives on one core.
    Greedy-pack sorted edges into 128-edge tiles spanning <=8 distinct dsts.
    Fold the constant MLP gates + all normalizations into the logit/value
    weights.  Upload only: bf16 node-feature table [N,128], per-edge indices
    + edge attrs (~5MB/core vs ~26MB/core for host-side gathers).
  - Device: gpsimd.dma_gather pulls src rows ([128e,128] bf16) and dst rows
    transposed ([128f,128e] bf16) straight from the HBM table.  Per-edge
    dense math -> logit[128,4]; softmax max-shift is dropped (|logit|<2, and
    alpha = g/z is shift-invariant per segment), so one pass suffices:
    g = cutoff*exp(logit), sq = sqrt(g), vals = [g | sq*val_s | sq*val_v].
    Per-tile segment-sum via one-hot PE matmul -> segs[t] = onehot.T @ vals
    ([8 slots, 196]).  Download only [NT,8,196] f32 per core.
  - Host: scatter tile/slot sums to nodes (each dst appears in exactly one
    tile/slot), ns /= sqrt(z), small output linears.
"""

import math
import numpy as np

N, E = 10000, 320000
F0, F1 = 32, 16
K = F0 + F1          # 48
H = 4
HID = 64
SQRT3 = math.sqrt(3.0)
FAN_SQRT = 48.0      # sqrt(F0*K + F1*K)
NCORES = 8
SLOTS = 8
TILE_E = 128
NT_MAX = 368         # measured max tiles/core = 362 for this input

LAST_EXEC_NS = None


def _gelu(x):
    return 0.5 * x * (1.0 + np.tanh(np.sqrt(2.0 / np.pi) * (x + 0.044715 * x ** 3)))


def _mlp_np(y0, W1, W2, W3):
    h = _gelu(y0 @ W1)
    h = _gelu(h @ W2 / np.sqrt(float(HID)))
    return h @ W3 / np.sqrt(float(HID))


def _build_nc():
    import concourse.bass as bass
    import concourse.bacc as bacc
    import concourse.mybir as mybir
    import concourse.tile as tile

    f32 = mybir.dt.float32
    bf16 = mybir.dt.bfloat16
    i16 = mybir.dt.int16
    X = mybir.AxisListType.X
    EXP = mybir.ActivationFunctionType.Exp
    EQ = mybir.AluOpType.is_equal

    nc = bacc.Bacc(None)
    table_d = nc.declare_dram_parameter("table", [N, 128], bf16, isOutput=False)
    sidx_d = nc.declare_dram_parameter("sidx", [NT_MAX, 128, 8], i16, isOutput=False)
    qidx_d = nc.declare_dram_parameter("qidx", [NT_MAX, 128, 8], i16, isOutput=False)
    r3_d = nc.declare_dram_parameter("r3", [NT_MAX, 128, 3], f32, isOutput=False)
    sqc_d = nc.declare_dram_parameter("sqc", [NT_MAX, 128, 1], f32, isOutput=False)
    rel_d = nc.declare_dram_parameter("rel", [NT_MAX, 128, 1], bf16, isOutput=False)
    WA_d = nc.declare_dram_parameter("WA", [64, 384], bf16, isOutput=False)
    WB_d = nc.declare_dram_parameter("WB", [80, 384], bf16, isOutput=False)
    gvs_d = nc.declare_dram_parameter("gvs", [128, 48], f32, isOutput=False)
    gvv_d = nc.declare_dram_parameter("gvv", [128, 144], f32, isOutput=False)
    cmp_d = nc.declare_dram_parameter("cmp", [128, 8], bf16, isOutput=False)
    segs_d = nc.declare_dram_parameter("segs", [NT_MAX, 8, 196], f32, isOutput=True)

    with tile.TileContext(nc) as tc:
        with (
            tc.tile_pool(name="const", bufs=1) as cp,
            tc.tile_pool(name="io", bufs=3) as iop,
            tc.tile_pool(name="work", bufs=3) as wp,
            tc.tile_pool(name="psum", bufs=2, space=bass.MemorySpace.PSUM) as pp,
        ):
            WA_t = cp.tile([64, 384], bf16, tag="wa")
            nc.sync.dma_start(WA_t[:], WA_d[:])
            WB_t = cp.tile([80, 384], bf16, tag="wb")
            nc.sync.dma_start(WB_t[:], WB_d[:])
            gvs_t = cp.tile([128, 48], f32, tag="gvs")
            nc.sync.dma_start(gvs_t[:], gvs_d[:])
            gvv_t = cp.tile([128, 144], f32, tag="gvv")
            nc.sync.dma_start(gvv_t[:], gvv_d[:])
            cmp_t = cp.tile([128, 8], bf16, tag="cmp")
            nc.sync.dma_start(cmp_t[:], cmp_d[:])

            for t in range(NT_MAX):
                sid = iop.tile([128, 8], i16, tag="sid")
                nc.sync.dma_start(sid[:], sidx_d[t])
                qid = iop.tile([128, 8], i16, tag="qid")
                nc.sync.dma_start(qid[:], qidx_d[t])
                rr = iop.tile([128, 3], f32, tag="rr")
                nc.sync.dma_start(rr[:], r3_d[t])
                sqcc = iop.tile([128, 1], f32, tag="sqcc")
                nc.sync.dma_start(sqcc[:], sqc_d[t])
                relc = iop.tile([128, 1], bf16, tag="relc")
                nc.sync.dma_start(relc[:], rel_d[t])

                srcF = iop.tile([128, 1, 128], bf16, tag="srcF")
                nc.gpsimd.dma_gather(srcF[:], table_d[:, :], sid[:], 128, 128, 128)
                qT = iop.tile([128, 1, 128], bf16, tag="qT")
                nc.gpsimd.dma_gather(
                    qT[:], table_d[:, :], qid[:], 128, 128, 128, transpose=True
                )

                # PA = [B0 | D0], PB = [D1 | D2] via block-diagonal weights
                # (lhsT base partition must be 0/32/64)
                PA = pp.tile([128, 384], f32, tag="PA")
                nc.tensor.matmul(PA[:], qT[0:64, 0, :], WA_t[:])
                PB = pp.tile([128, 384], f32, tag="PB")
                nc.tensor.matmul(PB[:], qT[0:80, 0, :], WB_t[:])
                D = [PA[:, 192:384], PB[:, 0:192], PB[:, 192:384]]

                # o1s = [src_s | dot(src_v, r)] (raw; norms folded into weights)
                o1s = wp.tile([128, 48], bf16, tag="o1s")
                nc.vector.tensor_copy(o1s[:, 0:32], srcF[:, 0, 0:32])
                dotv = wp.tile([128, 16, 3], f32, tag="dotv")
                sv3 = srcF[:, 0, 80:128].rearrange("p (f c) -> p f c", c=3)
                rr_b16 = rr[:].rearrange("p c -> p () c").to_broadcast((128, 16, 3))
                nc.vector.tensor_mul(dotv[:], sv3, rr_b16)
                with nc.allow_low_precision(reason="3-elem dot to bf16"):
                    nc.vector.reduce_sum(o1s[:, 32:48], dotv[:], axis=X)

                # o1v = [src_v | src_s x r]  layout (j, c) j-major
                o1v = wp.tile([128, 144], bf16, tag="o1v")
                nc.vector.tensor_copy(o1v[:, 0:48], srcF[:, 0, 80:128])
                ss_b = srcF[:, 0, 0:32].rearrange("p f -> p f ()").to_broadcast((128, 32, 3))
                rr_b32 = rr[:].rearrange("p c -> p () c").to_broadcast((128, 32, 3))
                nc.vector.tensor_mul(
                    o1v[:, 48:144].rearrange("p (f c) -> p f c", c=3), ss_b, rr_b32
                )

                # logit[e,h] = sum_j B0*o1s + sum_{c,j} D_c*o1v_c
                Tt = wp.tile([128, 768], f32, tag="Tt")
                Tt3 = Tt[:].rearrange("p (h j) -> p h j", h=4)
                B03 = PA[:, 0:192].rearrange("p (h j) -> p h j", h=4)
                o1s_b = o1s[:].rearrange("p j -> p () j").to_broadcast((128, 4, 48))
                nc.vector.tensor_mul(Tt3[:, :, 0:48], B03, o1s_b)
                o1v3 = o1v[:].rearrange("p (j c) -> p j c", c=3)
                for c in range(3):
                    Dc3 = D[c].rearrange("p (h j) -> p h j", h=4)
                    o1vc_b = (
                        o1v3[:, :, c].rearrange("p j -> p () j").to_broadcast((128, 4, 48))
                    )
                    nc.vector.tensor_mul(Tt3[:, :, 48 + 48 * c:96 + 48 * c], Dc3, o1vc_b)
                lg = wp.tile([128, 4], f32, tag="lg")
                nc.vector.reduce_sum(lg[:], Tt3, axis=X)

                # sq = sqrt(cutoff)*exp(logit/2), g = sq^2
                ex = wp.tile([128, 4], f32, tag="ex")
                nc.scalar.activation(ex[:], lg[:], EXP, 0.0, 0.5)
                sq = wp.tile([128, 4], f32, tag="sq")
                nc.vector.tensor_mul(sq[:], ex[:], sqcc[:].to_broadcast((128, 4)))
                g = wp.tile([128, 4], f32, tag="g")
                nc.vector.tensor_mul(g[:], sq[:], sq[:])

                # vals = [g | sq*(o1s*gvs) | sq*(o1v*gvv)]  (bf16 for PE)
                vals = wp.tile([128, 196], bf16, tag="vals")
                nc.vector.tensor_copy(vals[:, 0:4], g[:])
                vst = wp.tile([128, 48], f32, tag="vst")
                nc.vector.tensor_mul(vst[:], o1s[:], gvs_t[:])
                sq_b12 = sq[:].rearrange("p h -> p h ()").to_broadcast((128, 4, 12))
                nc.vector.tensor_mul(
                    vals[:, 4:52].rearrange("p (h k) -> p h k", k=12),
                    vst[:].rearrange("p (h k) -> p h k", k=12),
                    sq_b12,
                )
                vvt = wp.tile([128, 144], f32, tag="vvt")
                nc.vector.tensor_mul(vvt[:], o1v[:], gvv_t[:])
                sq_b36 = sq[:].rearrange("p h -> p h ()").to_broadcast((128, 4, 36))
                nc.vector.tensor_mul(
                    vals[:, 52:196].rearrange("p (h k) -> p h k", k=36),
                    vvt[:].rearrange("p (h k) -> p h k", k=36),
                    sq_b36,
                )

                # one-hot segment sum: segs[t] = onehot.T @ vals
                oh = wp.tile([128, 8], bf16, tag="oh")
                nc.vector.tensor_tensor(oh[:], relc[:].to_broadcast((128, 8)), cmp_t[:], EQ)
                seg = pp.tile([8, 196], f32, tag="seg")
                nc.tensor.matmul(seg[:], oh[:], vals[:])
                segS = wp.tile([8, 196], f32, tag="segS")
                nc.scalar.copy(segS[:], seg[:])
                nc.sync.dma_start(segs_d[t], segS[:])
    nc.compile()
    return nc


_NC_CACHE = None


def _pack(edge_dst):
    """Sort edges by dst, split into 8 dst-ranges of ~E/8 edges, greedy-pack
    128-edge/8-slot tiles. Returns (order, per-core metadata)."""
    order = np.argsort(edge_dst, kind="stable")
    counts = np.bincount(edge_dst, minlength=N)
    cum = np.concatenate([[0], np.cumsum(counts)])
    bnd = [0]
    for c in range(1, NCORES):
        bnd.append(int(np.searchsorted(cum, c * E // NCORES)))
    bnd.append(N)
    cores = []
    for c in range(NCORES):
        tiles = []
        cur, cur_e = [], 0
        for d in range(bnd[c], bnd[c + 1]):
            cnt = int(counts[d])
            if cnt == 0:
                continue
            assert cnt <= TILE_E
            if cur_e + cnt > TILE_E or len(cur) == SLOTS:
                tiles.append(cur)
                cur, cur_e = [], 0
            cur.append((d, cnt))
            cur_e += cnt
        if cur:
            tiles.append(cur)
        nt = len(tiles)
        assert nt <= NT_MAX, f"core {c}: {nt} tiles > NT_MAX={NT_MAX}"
        perm = np.zeros((nt, TILE_E), dtype=np.int64)
        valid = np.zeros((nt, TILE_E), dtype=bool)
        rel = np.zeros((nt, TILE_E), dtype=np.int64)
        slot_dst = np.full((nt, SLOTS), -1, dtype=np.int64)
        for t, tl in enumerate(tiles):
            p = 0
            for s, (d, cnt) in enumerate(tl):
                perm[t, p:p + cnt] = np.arange(cum[d], cum[d] + cnt)
                rel[t, p:p + cnt] = s
                valid[t, p:p + cnt] = True
                slot_dst[t, s] = d
                p += cnt
        cores.append(dict(nt=nt, perm=perm, valid=valid, rel=rel, slot_dst=slot_dst))
    return order, cores


def _wrap_idx(idx):
    """[nt,128] int -> dma_gather wrapped layout [nt,128,8] int16
    (pos i at [i%16, i//16], replicated across the 8 groups of 16)."""
    nt = idx.shape[0]
    w = idx.reshape(nt, 8, 16).transpose(0, 2, 1).astype(np.int16)  # [nt,16,8]
    return np.ascontiguousarray(np.tile(w, (1, 8, 1)))              # [nt,128,8]


def kernel(edge_src, edge_dst, edge_weight_cutoff, edge_attr_s, edge_attr_v,
           node_s, node_v, Wk1, Wk2, Wk3, Wv1, Wv2, Wv3, Wlog0, Wlog1,
           Wout0, Wout1):
    global LAST_EXEC_NS, _NC_CACHE
    import ml_dtypes
    from concourse.bass_utils import run_bass_kernel_spmd

    f32 = np.float32
    bf16 = ml_dtypes.bfloat16
    edge_src = np.asarray(edge_src).astype(np.int64)
    edge_dst = np.asarray(edge_dst).astype(np.int64)
    cut_all = np.asarray(edge_weight_cutoff, dtype=f32)
    ea_s = np.asarray(edge_attr_s, dtype=f32)
    r_all = np.asarray(edge_attr_v, dtype=f32)
    node_s = np.asarray(node_s, dtype=f32)
    node_v = np.asarray(node_v, dtype=f32)

    # fold constant MLP gates (edge_attr_s is constant) + norms into weights
    u = np.unique(ea_s[:, 0])
    assert u.size == 1, "non-constant edge_attr_s unsupported by device path"
    y0 = u.reshape(1, 1).astype(np.float64)
    gk = _mlp_np(y0, np.asarray(Wk1, np.float64), np.asarray(Wk2, np.float64),
                 np.asarray(Wk3, np.float64))[0]
    gv = _mlp_np(y0, np.asarray(Wv1, np.float64), np.asarray(Wv2, np.float64),
                 np.asarray(Wv3, np.float64))[0]
    scale = 1.0 / FAN_SQRT
    jfac = np.where(np.arange(K) >= F0, 1.0 / SQRT3, 1.0)
    W0f = (np.asarray(Wlog0, np.float64).transpose(0, 2, 1)
           * (gk[:K] * jfac * scale)[None, None, :]).reshape(F0, H * K)
    W1f = (np.asarray(Wlog1, np.float64).transpose(0, 2, 1)
           * (gk[K:] * scale / SQRT3)[None, None, :]).reshape(F1, H * K)
    gvs = (gv[:K] * jfac).astype(f32)
    gvv = np.repeat(gv[K:], 3).astype(f32)

    # node table [N,128]: s | v_c0 | v_c1 | v_c2 | v i-major
    table = np.zeros((N, 128), f32)
    table[:, 0:32] = node_s
    for c in range(3):
        table[:, 32 + 16 * c:48 + 16 * c] = node_v[:, :, c]
    table[:, 80:128] = node_v.reshape(N, 48)
    table_b = table.astype(bf16)

    order, cores = _pack(edge_dst)
    WA = np.zeros((64, 384), np.float64)
    WA[0:32, 0:192] = W0f
    WA[32:48, 192:384] = W1f
    WB = np.zeros((80, 384), np.float64)
    WB[48:64, 0:192] = W1f
    WB[64:80, 192:384] = W1f
    WA_b = np.ascontiguousarray(WA.astype(bf16))
    WB_b = np.ascontiguousarray(WB.astype(bf16))
    gvs_u = np.ascontiguousarray(np.broadcast_to(gvs[None, :], (128, K)))
    gvv_u = np.ascontiguousarray(np.broadcast_to(gvv[None, :], (128, 144)))
    cmp_u = np.ascontiguousarray(
        np.broadcast_to(np.arange(SLOTS, dtype=f32)[None, :], (128, SLOTS))
    ).astype(bf16)

    sqrt_cut = np.sqrt(cut_all)
    in_maps = []
    for c in range(NCORES):
        C = cores[c]
        nt = C["nt"]
        perm, valid, rel = C["perm"], C["valid"], C["rel"]
        eidx = order[np.clip(perm, 0, E - 1)]
        sidx = np.where(valid, edge_src[eidx], 0)
        qidx = np.where(valid, edge_dst[eidx], 0)
        r3 = np.where(valid[..., None], r_all[eidx], 0.0).astype(f32)
        sqc = np.where(valid, sqrt_cut[eidx], 0.0).astype(f32)

        def padnt(a):
            pad = NT_MAX - a.shape[0]
            if pad == 0:
                return np.ascontiguousarray(a)
            return np.concatenate([a, np.zeros((pad,) + a.shape[1:], a.dtype)], 0)

        in_maps.append(dict(
            table=table_b,
            sidx=padnt(_wrap_idx(sidx)),
            qidx=padnt(_wrap_idx(qidx)),
            r3=padnt(r3),
            sqc=padnt(sqc[..., None]),
            rel=padnt(rel.astype(bf16)[..., None]),
            WA=WA_b, WB=WB_b, gvs=gvs_u, gvv=gvv_u, cmp=cmp_u,
        ))

    if _NC_CACHE is None:
        _NC_CACHE = _build_nc()
    import time as _time
    _t0 = _time.time()
    res = run_bass_kernel_spmd(_NC_CACHE, in_maps, core_ids=list(range(NCORES)))
    LAST_EXEC_NS = res.exec_time_ns
    if LAST_EXEC_NS is None:  # no NTFF hook in this container: wall-clock proxy
        LAST_EXEC_NS = int((_time.time() - _t0) * 1e9)

    # host: scatter tile/slot segment sums to nodes, finish
    Z = np.zeros((N, H), f32)
    NS = np.zeros((N, K), f32)
    NV = np.zeros((N, 144), f32)
    for c in range(NCORES):
        C = cores[c]
        segs = res.results[c]["segs"][:C["nt"]]        # [nt,8,196]
        sd = C["slot_dst"].reshape(-1)
        ok = sd >= 0
        flat = segs.reshape(-1, 4 + K + 144)[ok]
        np.add.at(Z, sd[ok], flat[:, 0:4])
        np.add.at(NS, sd[ok], flat[:, 4:4 + K])
        np.add.at(NV, sd[ok], flat[:, 4 + K:])

    Zs = np.where(Z == 0.0, 1.0, Z)
    rz = 1.0 / np.sqrt(Zs)
    ns = NS * rz[:, np.arange(K) // 12]
    nv = NV * rz[:, np.arange(144) // 36]
    out_s = ns @ np.asarray(Wout0, f32) / np.sqrt(float(K))
    out_v = np.einsum("nfc,fg->ngc", nv.reshape(N, K, 3),
                      np.asarray(Wout1, f32)) / np.sqrt(float(K))
    return np.concatenate([out_s, out_v.reshape(N, 48)], axis=1).astype(f32)


# revision 26
# speedup vs baseline: 33.5928x; 10.1696x over previous
"""EquivariantTransformerBlock on 8 TRN2 NeuronCores.

Strategy (v4: minimize wire bytes AND instruction count; the axon tunnel
moves ~30MB/s and per-instruction issue overhead dominates device exec):
  - Host: sort edges by dst, split dsts into 8 contiguous ranges (~E/8 edges
    each) -> one range per core, so each dst's segment lives wholly on one
    core.  Greedy-pack sorted edges into 128-edge tiles spanning <=8 distinct
    dsts.  Fold the constant MLP gates + all normalizations into the
    logit/value weights.  Upload per core: one bf16 node-table shard
    [N/8,128] (AllGathered on device), an int16 index stream, packed bf16
    edge attrs.
  - Device: process TB=4 tiles per batch.  One gpsimd.dma_gather per batch
    pulls 512 src rows ([128e,4,128] bf16) and 512 dst rows transposed
    ([128f,512e] bf16) from the HBM table.  Per-edge dense math ->
    logit[e,4]; the softmax max-shift is dropped (|logit|<2, alpha = g/z is
    shift-invariant per segment), so one pass suffices:
    sq = sqrt(cutoff)*exp(logit/2), vals = [sq^2 | sq*val_s | sq*val_v].
    Per-tile segment-sum via one-hot PE matmul -> segs = onehot.T @ vals
    ([32 slots, 196] accumulated over the 4 tiles), then one dma_scatter_add
    writes each slot row to its compact per-dst output row.  Download only
    [1296, 256] bf16 per core.
  - Host: slice compact rows into Z/NS/NV, ns /= sqrt(z), output linears.
"""

import math
import numpy as np

N, E = 10000, 320000
F0, F1 = 32, 16
K = F0 + F1          # 48
H = 4
HID = 64
SQRT3 = math.sqrt(3.0)
FAN_SQRT = 48.0      # sqrt(F0*K + F1*K)
NCORES = 8
SLOTS = 8
TILE_E = 128
TB = 4               # tiles per device batch
NT_MAX = 376         # measured max tiles/core = 362 for this input (+margin)
NB = NT_MAX // TB    # 94
NROWS = 1296         # compact output rows (>= N/NCORES, +dump row 1280)
DUMP = 1280

LAST_EXEC_NS = None


def _gelu(x):
    return 0.5 * x * (1.0 + np.tanh(np.sqrt(2.0 / np.pi) * (x + 0.044715 * x ** 3)))


def _mlp_np(y0, W1, W2, W3):
    h = _gelu(y0 @ W1)
    h = _gelu(h @ W2 / np.sqrt(float(HID)))
    return h @ W3 / np.sqrt(float(HID))


def _build_nc():
    import concourse.bass as bass
    import concourse.bacc as bacc
    import concourse.mybir as mybir
    import concourse.tile as tile

    f32 = mybir.dt.float32
    bf16 = mybir.dt.bfloat16
    i16 = mybir.dt.int16
    X = mybir.AxisListType.X
    EXP = mybir.ActivationFunctionType.Exp
    EQ = mybir.AluOpType.is_equal

    nc = bacc.Bacc(None)
    NSH = N // NCORES
    tshard_d = nc.declare_dram_parameter("tshard", [NSH, 128], bf16, isOutput=False)
    idx_d = nc.declare_dram_parameter("idx", [16, NB * 64], i16, isOutput=False)
    ea_d = nc.declare_dram_parameter("ea", [NB, 128, TB, 5], bf16, isOutput=False)
    WA_d = nc.declare_dram_parameter("WA", [64, 384], bf16, isOutput=False)
    WB_d = nc.declare_dram_parameter("WB", [80, 384], bf16, isOutput=False)
    gvs_d = nc.declare_dram_parameter("gvs", [128, 48], f32, isOutput=False)
    gvv_d = nc.declare_dram_parameter("gvv", [128, 144], f32, isOutput=False)
    cmp_d = nc.declare_dram_parameter("cmp", [128, 32], bf16, isOutput=False)
    sidx2_d = nc.declare_dram_parameter("sidx2", [16, NB * 2], i16, isOutput=False)
    segs_d = nc.declare_dram_parameter("segs", [NROWS, 256], bf16, isOutput=True)

    with tile.TileContext(nc) as tc:
        with (
            tc.tile_pool(name="const", bufs=1) as cp,
            tc.tile_pool(name="dram", bufs=1, space="DRAM") as dp,
            tc.tile_pool(name="io", bufs=3) as iop,
            tc.tile_pool(name="work", bufs=3) as wp,
            tc.tile_pool(name="psum", bufs=2, space=bass.MemorySpace.PSUM) as pp,
        ):
            # AllGather the bf16 node table: each core uploads N/8 rows
            shard_b = dp.tile([NSH, 128], bf16, tag="shard_b")
            table_t = dp.tile([N, 128], bf16, tag="table")
            nc.gpsimd.dma_start(shard_b[:], tshard_d[:])
            nc.gpsimd.collective_compute(
                "AllGather",
                mybir.AluOpType.bypass,
                replica_groups=[list(range(NCORES))],
                ins=[shard_b.opt()],
                outs=[table_t.opt()],
            )
            WA_t = cp.tile([64, 384], bf16, tag="wa")
            nc.sync.dma_start(WA_t[:], WA_d[:])
            WB_t = cp.tile([80, 384], bf16, tag="wb")
            nc.sync.dma_start(WB_t[:], WB_d[:])
            gvs_t = cp.tile([128, 48], f32, tag="gvs")
            nc.sync.dma_start(gvs_t[:], gvs_d[:])
            gvv_t = cp.tile([128, 144], f32, tag="gvv")
            nc.sync.dma_start(gvv_t[:], gvv_d[:])
            cmp_t = cp.tile([128, 32], bf16, tag="cmp")
            nc.sync.dma_start(cmp_t[:], cmp_d[:])
            # index stream: load [16, NB*64] once, replicate to all 8
            # partition groups (dma_gather wants indices repeated per group)
            idxb = cp.tile([128, NB * 64], i16, tag="idxb")
            nc.sync.dma_start(idxb[0:16, :], idx_d[:])
            idxc = cp.tile([128, NB * 2], i16, tag="idxc")
            nc.sync.dma_start(idxc[0:16, :], sidx2_d[:])
            for r in range(1, 8):
                nc.sync.dma_start(idxb[16 * r:16 * r + 16, :], idx_d[:])
                nc.sync.dma_start(idxc[16 * r:16 * r + 16, :], sidx2_d[:])

            for b in range(NB):
                ea = iop.tile([128, TB, 5], bf16, tag="ea")
                nc.sync.dma_start(ea[:], ea_d[b])

                srcF = iop.tile([128, TB, 128], bf16, tag="srcF")
                nc.gpsimd.dma_gather(
                    srcF[:], table_t[:, :], idxb[:, 64 * b:64 * b + 32],
                    TB * 128, TB * 128, 128,
                )
                qT = iop.tile([128, 1, TB * 128], bf16, tag="qT")
                nc.gpsimd.dma_gather(
                    qT[:], table_t[:, :], idxb[:, 64 * b + 32:64 * b + 64],
                    TB * 128, TB * 128, 128, transpose=True,
                )

                # per tile: PA = [B0 | D0], PB = [D1 | D2] (block-diag weights;
                # lhsT/rhs base partition must be 0/32/64); copy to Pcat on the
                # scalar engine so the logit products batch into one DVE mul
                Pcat = wp.tile([128, TB, 768], f32, tag="Pcat")
                for tb in range(TB):
                    qTs = qT[:, 0, 128 * tb:128 * tb + 128]
                    PA = pp.tile([128, 384], f32, tag="PA")
                    nc.tensor.matmul(PA[:], qTs[0:64, :], WA_t[:])
                    PB = pp.tile([128, 384], f32, tag="PB")
                    nc.tensor.matmul(PB[:], qTs[0:80, :], WB_t[:])
                    nc.scalar.copy(Pcat[:, tb, 0:384], PA[:])
                    nc.scalar.copy(Pcat[:, tb, 384:768], PB[:])

                # o1cat [128, TB, 4, 48]: group 0 = o1s = [s | dot(v,r)],
                # groups 1+c = o1v_c = [v_c | s*r_c]
                rr = ea[:, :, 0:3]                       # [128, TB, 3]
                o1cat = wp.tile([128, TB, 4, 48], bf16, tag="o1cat")
                nc.vector.tensor_copy(o1cat[:, :, 0, 0:32], srcF[:, :, 0:32])
                dotv = wp.tile([128, TB, 16, 3], f32, tag="dotv")
                sv3 = srcF[:, :, 80:128].rearrange("p t (f c) -> p t f c", c=3)
                rr_b16 = rr.rearrange("p t c -> p t () c").to_broadcast((128, TB, 16, 3))
                nc.vector.tensor_mul(dotv[:], sv3, rr_b16)
                with nc.allow_low_precision(reason="3-elem dot to bf16"):
                    nc.vector.reduce_sum(o1cat[:, :, 0, 32:48], dotv[:], axis=X)
                # v in c-major blocks: srcF cols 32:80 viewed [TB, 3, 16]
                nc.vector.tensor_copy(
                    o1cat[:, :, 1:4, 0:16],
                    srcF[:, :, 32:80].rearrange("p t (c f) -> p t c f", c=3),
                )
                ss_b = (srcF[:, :, 0:32].rearrange("p t f -> p t () f")
                        .to_broadcast((128, TB, 3, 32)))
                rr_b32 = (rr.rearrange("p t c -> p t c ()")
                          .to_broadcast((128, TB, 3, 32)))
                nc.vector.tensor_mul(o1cat[:, :, 1:4, 16:48], ss_b, rr_b32)

                # logit: Tt[p,t,g,h,j] = Pcat[p,t,(g,h,j)] * o1cat[p,t,g,j]
                Tt = wp.tile([128, TB, 768], f32, tag="Tt")
                Tt5 = Tt[:].rearrange("p t (g h j) -> p t g h j", g=4, h=4)
                P5 = Pcat[:].rearrange("p t (g h j) -> p t g h j", g=4, h=4)
                o1_b = (o1cat[:].rearrange("p t g j -> p t g () j")
                        .to_broadcast((128, TB, 4, 4, 48)))
                nc.vector.tensor_mul(Tt5, P5, o1_b)
                lgg = wp.tile([128, TB, 4, 4], f32, tag="lgg")   # [.., g, h]
                nc.vector.reduce_sum(lgg[:], Tt5, axis=X)
                lg = wp.tile([128, TB, 4], f32, tag="lg")        # sum over g
                nc.vector.reduce_sum(
                    lg[:], lgg[:].rearrange("p t g h -> p t h g"), axis=X
                )

                # sq = sqrt(cutoff)*exp(logit/2), g2 = sq^2
                ex = wp.tile([128, TB, 4], f32, tag="ex")
                nc.scalar.activation(ex[:], lg[:], EXP, 0.0, 0.5)
                sq = wp.tile([128, TB, 4], f32, tag="sq")
                nc.vector.tensor_mul(
                    sq[:], ex[:],
                    ea[:, :, 3:4].to_broadcast((128, TB, 4)),
                )
                g2 = wp.tile([128, TB, 4], f32, tag="g2")
                nc.vector.tensor_mul(g2[:], sq[:], sq[:])

                # vals[p,t,196] = [g2 | sq*(o1s*gvs) | sq*(o1v*gvv)] (bf16)
                vals = wp.tile([128, TB, 196], bf16, tag="vals")
                nc.vector.tensor_copy(vals[:, :, 0:4], g2[:])
                vst = wp.tile([128, TB, 48], f32, tag="vst")
                gvs_b = gvs_t[:].rearrange("p j -> p () j").to_broadcast((128, TB, 48))
                nc.vector.tensor_mul(vst[:], o1cat[:, :, 0, :], gvs_b)
                sq_b12 = (sq[:].rearrange("p t h -> p t h ()")
                          .to_broadcast((128, TB, 4, 12)))
                nc.vector.tensor_mul(
                    vals[:, :, 4:52].rearrange("p t (h k) -> p t h k", k=12),
                    vst[:].rearrange("p t (h k) -> p t h k", k=12),
                    sq_b12,
                )
                # o1v in (j,c) j-major = o1cat groups 1:4 with axes swapped
                vvt = wp.tile([128, TB, 48, 3], f32, tag="vvt")
                o1v_jc = o1cat[:, :, 1:4, :].rearrange("p t c j -> p t j c")
                gvv_b = (gvv_t[:].rearrange("p (j c) -> p () j c", c=3)
                         .to_broadcast((128, TB, 48, 3)))
                nc.vector.tensor_mul(vvt[:], o1v_jc, gvv_b)
                sq_b36 = (sq[:].rearrange("p t h -> p t h () ()")
                          .to_broadcast((128, TB, 4, 12, 3)))
                nc.vector.tensor_mul(
                    vals[:, :, 52:196].rearrange("p t (h x c) -> p t h x c", h=4, x=12),
                    vvt[:].rearrange("p t (h x) c -> p t h x c", h=4),
                    sq_b36,
                )

                # one-hot segment sums: segs[b,:,tb,:] = onehot_tb.T @ vals_tb
                # one-hot over (tb*8+slot); slot ids in ea[...,4] are
                # pre-offset by 8*tb on the host, so all 4 tiles accumulate
                # into one [32,196] PSUM tile, then one scatter to compact rows
                oh = wp.tile([128, TB, 32], bf16, tag="oh")
                nc.vector.tensor_tensor(
                    oh[:], ea[:, :, 4:5].to_broadcast((128, TB, 32)),
                    cmp_t[:].rearrange("p s -> p () s").to_broadcast((128, TB, 32)),
                    EQ,
                )
                seg32 = pp.tile([32, 196], f32, tag="seg")
                for tb in range(TB):
                    nc.tensor.matmul(seg32[:], oh[:, tb, :], vals[:, tb, :],
                                     start=(tb == 0), stop=(tb == TB - 1))
                stage = wp.tile([32, 4, 196], bf16, tag="segS")
                nc.scalar.copy(stage[:, 0, :], seg32[:])
                nc.gpsimd.dma_scatter_add(
                    segs_d[:, 0:196], stage[:], idxc[:, 2 * b:2 * b + 2],
                    32, 32, 196, elem_step=256,
                )
    nc.compile()
    return nc


_NC_CACHE = None


def _pack(edge_dst):
    """Sort edges by dst, split into 8 dst-ranges of ~E/8 edges, greedy-pack
    128-edge/8-slot tiles. Returns (order, per-core metadata) with flat
    per-edge scatter positions (vectorized assembly)."""
    order = np.argsort(edge_dst, kind="stable")
    counts = np.bincount(edge_dst, minlength=N)
    cum = np.concatenate([[0], np.cumsum(counts)])
    bnd = [0]
    for c in range(1, NCORES):
        bnd.append(int(np.searchsorted(cum, c * E // NCORES)))
    bnd.append(N)
    cores = []
    for c in range(NCORES):
        d_lo, d_hi = bnd[c], bnd[c + 1]
        cnts = counts[d_lo:d_hi]
        nz = np.nonzero(cnts)[0]
        sz = cnts[nz].astype(np.int64)
        n_d = len(nz)
        t_arr = np.empty(n_d, np.int64)
        s_arr = np.empty(n_d, np.int64)
        p_arr = np.empty(n_d, np.int64)
        t, s, p = 0, 0, 0
        for i, csize in enumerate(sz.tolist()):
            assert csize <= TILE_E
            if p + csize > TILE_E or s == SLOTS:
                t += 1
                s = 0
                p = 0
            t_arr[i] = t
            s_arr[i] = s
            p_arr[i] = p
            s += 1
            p += csize
        nt = t + 1
        assert nt <= NT_MAX, f"core {c}: {nt} tiles > NT_MAX={NT_MAX}"
        e_lo, e_hi = int(cum[d_lo]), int(cum[d_hi])
        rep_t = np.repeat(t_arr, sz)
        rep_s = np.repeat(s_arr, sz)
        offs = np.arange(e_hi - e_lo) - np.repeat(cum[d_lo + nz] - e_lo, sz)
        flatpos = rep_t * TILE_E + np.repeat(p_arr, sz) + offs
        sd_local = np.full((NT_MAX, SLOTS), DUMP, np.int64)
        sd_local[t_arr, s_arr] = nz
        cores.append(dict(nt=nt, flatpos=flatpos, rel=rep_s + SLOTS * (rep_t % TB),
                          e_lo=e_lo, e_hi=e_hi, sd_local=sd_local,
                          d_lo=d_lo, d_hi=d_hi))
    return order, cores


def _wrap16(idx):
    """[nb, 512] int -> dma_gather wrapped layout [nb,16,32] int16
    (pos i at [i%16, i//16])."""
    nb = idx.shape[0]
    return idx.reshape(nb, 32, 16).transpose(0, 2, 1).astype(np.int16)


def kernel(edge_src, edge_dst, edge_weight_cutoff, edge_attr_s, edge_attr_v,
           node_s, node_v, Wk1, Wk2, Wk3, Wv1, Wv2, Wv3, Wlog0, Wlog1,
           Wout0, Wout1):
    global LAST_EXEC_NS, _NC_CACHE
    import ml_dtypes
    from concourse.bass_utils import run_bass_kernel_spmd

    f32 = np.float32
    bf16 = ml_dtypes.bfloat16
    edge_src = np.asarray(edge_src).astype(np.int64)
    edge_dst = np.asarray(edge_dst).astype(np.int64)
    cut_all = np.asarray(edge_weight_cutoff, dtype=f32)
    ea_s = np.asarray(edge_attr_s, dtype=f32)
    r_all = np.asarray(edge_attr_v, dtype=f32)
    node_s = np.asarray(node_s, dtype=f32)
    node_v = np.asarray(node_v, dtype=f32)

    # fold constant MLP gates (edge_attr_s is constant) + norms into weights
    u = np.unique(ea_s[:, 0])
    assert u.size == 1, "non-constant edge_attr_s unsupported by device path"
    y0 = u.reshape(1, 1).astype(np.float64)
    gk = _mlp_np(y0, np.asarray(Wk1, np.float64), np.asarray(Wk2, np.float64),
                 np.asarray(Wk3, np.float64))[0]
    gv = _mlp_np(y0, np.asarray(Wv1, np.float64), np.asarray(Wv2, np.float64),
                 np.asarray(Wv3, np.float64))[0]
    scale = 1.0 / FAN_SQRT
    jfac = np.where(np.arange(K) >= F0, 1.0 / SQRT3, 1.0)
    W0f = (np.asarray(Wlog0, np.float64).transpose(0, 2, 1)
           * (gk[:K] * jfac * scale)[None, None, :]).reshape(F0, H * K)
    W1f = (np.asarray(Wlog1, np.float64).transpose(0, 2, 1)
           * (gk[K:] * scale / SQRT3)[None, None, :]).reshape(F1, H * K)
    gvs = (gv[:K] * jfac).astype(f32)
    gvv = np.repeat(gv[K:], 3).astype(f32)

    # node table [N,128]: s | v_c0 | v_c1 | v_c2 | v i-major
    table = np.zeros((N, 128), f32)
    table[:, 0:32] = node_s
    for c in range(3):
        table[:, 32 + 16 * c:48 + 16 * c] = node_v[:, :, c]
    table[:, 80:128] = node_v.reshape(N, 48)
    table_b = table.astype(bf16)

    order, cores = _pack(edge_dst)
    WA = np.zeros((64, 384), np.float64)
    WA[0:32, 0:192] = W0f
    WA[32:48, 192:384] = W1f
    WB = np.zeros((80, 384), np.float64)
    WB[48:64, 0:192] = W1f
    WB[64:80, 192:384] = W1f
    WA_b = np.ascontiguousarray(WA.astype(bf16))
    WB_b = np.ascontiguousarray(WB.astype(bf16))
    gvs_u = np.ascontiguousarray(np.broadcast_to(gvs[None, :], (128, K)))
    gvv_u = np.ascontiguousarray(np.broadcast_to(gvv[None, :], (128, 144)))
    cmp_u = np.ascontiguousarray(
        np.broadcast_to(np.arange(TB * SLOTS, dtype=f32)[None, :], (128, TB * SLOTS))
    ).astype(bf16)

    sqrt_cut = np.sqrt(cut_all)
    src_sorted = edge_src[order]
    dst_sorted = edge_dst[order]
    r_sorted = r_all[order]
    sqc_sorted = sqrt_cut[order]
    NSH = N // NCORES
    L = NT_MAX * TILE_E
    in_maps = []
    for c in range(NCORES):
        C = cores[c]
        fp = C["flatpos"]
        sl = slice(C["e_lo"], C["e_hi"])

        sidx_f = np.zeros(L, np.int16)
        sidx_f[fp] = src_sorted[sl].astype(np.int16)
        qidx_f = np.zeros(L, np.int16)
        qidx_f[fp] = dst_sorted[sl].astype(np.int16)
        # index stream [16, NB*64]: per batch 32 cols src idx then 32 cols dst
        sid_b = _wrap16(sidx_f.reshape(NB, TB * TILE_E))
        qid_b = _wrap16(qidx_f.reshape(NB, TB * TILE_E))
        idx16 = np.concatenate([sid_b, qid_b], axis=2)     # [NB,16,64]
        idx_u = np.ascontiguousarray(
            idx16.transpose(1, 0, 2).reshape(16, NB * 64))

        # packed edge attrs [NB,128,TB,5]: r3 | sqrt(cutoff) | slot(+8*tb)
        ea_f = np.zeros((L, 5), f32)
        ea_f[fp, 0:3] = r_sorted[sl]
        ea_f[fp, 3] = sqc_sorted[sl]
        ea_f[fp, 4] = C["rel"]
        ea_u = np.ascontiguousarray(
            ea_f.reshape(NB, TB, TILE_E, 5).transpose(0, 2, 1, 3).astype(bf16))

        # scatter index stream: compact row per (tile,slot), DUMP for unused
        s2 = C["sd_local"].reshape(NB, TB * SLOTS)               # [NB,32]
        s2w = s2.reshape(NB, 2, 16).transpose(0, 2, 1).astype(np.int16)
        sidx2_u = np.ascontiguousarray(s2w.transpose(1, 0, 2).reshape(16, NB * 2))

        in_maps.append(dict(
            tshard=np.ascontiguousarray(table_b[c * NSH:(c + 1) * NSH]),
            idx=idx_u,
            ea=ea_u,
            sidx2=sidx2_u,
            WA=WA_b, WB=WB_b, gvs=gvs_u, gvv=gvv_u, cmp=cmp_u,
        ))

    if _NC_CACHE is None:
        _NC_CACHE = _build_nc()
    import time as _time
    _t0 = _time.time()
    res = run_bass_kernel_spmd(_NC_CACHE, in_maps, core_ids=list(range(NCORES)))
    LAST_EXEC_NS = res.exec_time_ns
    if LAST_EXEC_NS is None:  # no NTFF hook in this container: wall-clock proxy
        LAST_EXEC_NS = int((_time.time() - _t0) * 1e9)

    # host: assemble compact per-dst rows (device scattered them already)
    Z = np.zeros((N, H), f32)
    NS = np.zeros((N, K), f32)
    NV = np.zeros((N, 144), f32)
    for c in range(NCORES):
        C = cores[c]
        nrows = C["d_hi"] - C["d_lo"]
        block = res.results[c]["segs"][:nrows, 0:196].astype(f32)  # [nrows,196]
        Z[C["d_lo"]:C["d_hi"]] = block[:, 0:4]
        NS[C["d_lo"]:C["d_hi"]] = block[:, 4:4 + K]
        NV[C["d_lo"]:C["d_hi"]] = block[:, 4 + K:]

    Zs = np.where(Z == 0.0, 1.0, Z)
    rz = 1.0 / np.sqrt(Zs)
    ns = NS * rz[:, np.arange(K) // 12]
    nv = NV * rz[:, np.arange(144) // 36]
    out_s = ns @ np.asarray(Wout0, f32) / np.sqrt(float(K))
    out_v = np.einsum("nfc,fg->ngc", nv.reshape(N, K, 3),
                      np.asarray(Wout1, f32)) / np.sqrt(float(K))
    return np.concatenate([out_s, out_v.reshape(N, 48)], axis=1).astype(f32)
